# revision 2
# baseline (speedup 1.0000x reference)
"""Causal single-head attention (B=4, T=4096, D=1024, H=64) on 8 TRN2 cores.

Sharding: core c -> batch b=c//2, parity p=c%2; core owns the 16 interleaved
query tiles {128*(2i+p)} of its batch (load-balances the causal triangle).
SPMD-uniform program; per-core differences live in input data only.

v3: all-bf16 + fine-grained causal trim + software-pipelined emission.
  * bf16 matmuls (1 cyc/row), fp32 PSUM accumulation
  * x streamed in 4 column waves of 512 tokens per half; wave w's [Wq|Wk]
    projection + natural-v run late in span w-1, its [Wk|Wv] (partner half)
    runs early in span w -> exp (ACT engine) starts ~8us in and never starves
  * S^T trimmed at 128x128 tile granularity: own/partner chunk c of span j
    only computes query tiles u >= c-4j; masking reduces to ONE 128x128
    tril tile (own diagonal) + ONE all-0/1 tile (partner diagonal, parity
    asymmetry carried in data)
  * PV in natural orientation: out[128q, 65] += P^T-chunk @ [V|1] -> 65
    cols/chunk-tile, sumexp rides in col 64, no output transposes
"""

import re
import numpy as np
import ml_dtypes

B, T, D, H = 4, 4096, 1024, 64
NT = T // 128           # 32 key tiles per batch
NOWN = NT // 2          # 16 query tiles per core
ND = D // 128           # 8 contraction tiles
NW = 4                  # column waves of 512 tokens per half

BF16 = ml_dtypes.bfloat16

_PROG = None
LAST_EXEC_TIME_NS = None
LAST_RESULTS = None


def _patch_tile_drain():
    """Walrus in this container allows only one sync-wait on NO_STRUCT
    instructions; TileContext's tail drain carries one wait per DMA lane.
    Split it into one drain per outstanding proc."""
    import bass_rust
    import concourse.tile as tile

    if getattr(tile.TileContext, "_drain_patched", False):
        return

    def _drain_and_barrier(self, tick_clock, wait_clock):
        nc = self.nc
        gvec = tick_clock.global_clock
        ticks = eval(re.match(r"VectorClock\((\[.*\])\)", repr(gvec)).group(1))
        for pr, tk in enumerate(ticks):
            if tk > 0:
                vec = [0] * len(ticks)
                vec[pr] = tk
                d = nc.sync.drain()
                wait_clock.add_sem_waits(
                    d.ins,
                    bass_rust.ScopedClock({None: bass_rust.VectorClock(vec)}),
                )
        nc.sync.drain()
        nc.all_engine_barrier()
        assert self.sems is not None
        popped = nc._tile_sem_poison_stack.pop()
        assert popped is self._sem_poison
        nc.clear_and_free_semaphores(list(self.sems.allocated().values()))
        nc.all_engine_barrier()

    tile.TileContext._drain_and_barrier = _drain_and_barrier
    tile.TileContext._drain_patched = True


def _split_multi_waits(nc):
    """This walrus build allows at most one sync-wait per instruction.
    Hoist extra waits onto injected same-engine NOPs placed just before the
    owning instruction (same engine stream => identical semantics)."""
    import bass_rust

    for bb in nc.main_func.blocks:
        new_list = []
        for ins in bb.instructions:
            si = ins.sync_info
            if si is not None and si.on_wait and len(si.on_wait) > 1:
                waits = list(si.on_wait)
                for w in waits[:-1]:
                    nop = nc.engines[ins.engine].nop().ins
                    for bb2 in nc.main_func.blocks:
                        if nop in bb2.instructions:
                            bb2.instructions.remove(nop)
                            break
                    nop.sync_info = bass_rust.SyncInfo(on_wait=[w], on_update=[])
                    new_list.append(nop)
                si.on_wait = [waits[-1]]
            new_list.append(ins)
        bb.instructions[:] = new_list


def _build_program():
    import concourse.bass as bass
    import concourse.tile as tile
    from concourse import mybir
    from concourse.masks import make_identity

    _patch_tile_drain()
    f32 = mybir.dt.float32
    bf16 = mybir.dt.bfloat16

    nc = bass.Bass()
    NJUNK = 26  # PE p-state warm-up matmuls (identity, no data deps)
    # per-wave x: [p, widx, dt*512]; widx = 2*w + half (0=own, 1=partner)
    xw = nc.dram_tensor("xw", [128, 2 * NW, ND * 512], bf16, kind="ExternalInput")
    wqk = nc.dram_tensor("wqk", [128, ND * 128], bf16, kind="ExternalInput")
    wkv = nc.dram_tensor("wkv", [128, ND * 128], bf16, kind="ExternalInput")
    wv = nc.dram_tensor("wv", [128, ND * H], bf16, kind="ExternalInput")
    masks = nc.dram_tensor("masks", [128, 2 * 128], bf16, kind="ExternalInput")
    out = nc.dram_tensor("out", [NOWN, 128, H], bf16, kind="ExternalOutput")

    with tile.TileContext(nc) as tc:
        with (
            tc.tile_pool(name="singles", bufs=1) as singles,
            tc.tile_pool(name="xwp", bufs=4) as xwp,
            tc.tile_pool(name="vtt", bufs=2) as vtt,
            tc.tile_pool(name="pp", bufs=3) as ppool,
            tc.tile_pool(name="rp", bufs=2) as rpool,
            tc.tile_pool(name="psW", bufs=1, space="PSUM") as psW,
            tc.tile_pool(name="psV", bufs=1, space="PSUM") as psV,
            tc.tile_pool(name="psS", bufs=1, space="PSUM") as psS,
            tc.tile_pool(name="psO0", bufs=1, space="PSUM") as psO0,
            tc.tile_pool(name="psO1", bufs=1, space="PSUM") as psO1,
        ):
            # ---- persistent state (DMAs are emitted further down, in
            # latency-critical order) ----
            wqk_sb = singles.tile([128, ND * 128], bf16)
            wkv_sb = singles.tile([128, ND * 128], bf16)
            wv_sb = singles.tile([128, ND * H], bf16)

            qT = singles.tile([64, T // 2], bf16)
            kT = singles.tile([64, T], bf16)             # own-first layout
            v_sb = singles.tile([128, NT, H + 1], bf16)  # natural v + ones col
            out_sb = singles.tile([128, NOWN, H], bf16)
            ident = singles.tile([128, 128], bf16)
            mask_sb = singles.tile([128, 2 * 128], bf16)

            make_identity(nc, ident)
            nc.vector.memset(v_sb[:, :, H : H + 1], 1.0)

            xo_t = [None] * NW
            xq_t = [None] * NW
            vo_t = [None] * NW

            def emit_wave_dma(w):
                xo = xwp.tile([128, ND * 512], bf16, tag="xw", name="xwt")
                nc.sync.dma_start(out=xo, in_=xw[:, 2 * w : 2 * w + 1, :])
                xq = xwp.tile([128, ND * 512], bf16, tag="xw", name="xwt")
                nc.sync.dma_start(out=xq, in_=xw[:, 2 * w + 1 : 2 * w + 2, :])
                xo_t[w], xq_t[w] = xo, xq

            def qk_units(w):
                """[Wq|Wk] over own wave w + natural-v for own tiles."""
                wsl = slice(w * 512, (w + 1) * 512)
                st = {}
                units = []

                def qk_d(d):
                    if d == 0:
                        st["qk"] = psW.tile([128, 512], f32, tag="wps", name="wps")
                    nc.tensor.matmul(st["qk"], lhsT=wqk_sb[:, d * 128 : (d + 1) * 128],
                                     rhs=xo_t[w][:, d * 512 : (d + 1) * 512],
                                     start=(d == 0), stop=(d == ND - 1))

                def qk_copy():
                    nc.vector.tensor_copy(out=qT[:, wsl], in_=st["qk"][0:64, :])
                    nc.vector.tensor_copy(out=kT[:, wsl], in_=st["qk"][64:128, :])

                def vo_u(u):
                    if u == 0:
                        vo_t[w] = psV.tile([128, 4, 128], f32, tag="vo", name="vo")
                    for d in range(ND):
                        nc.tensor.matmul(
                            vo_t[w][:, u, 0:H],
                            lhsT=xo_t[w][:, d * 512 + u * 128 : d * 512 + (u + 1) * 128],
                            rhs=wv_sb[:, d * H : (d + 1) * H],
                            start=(d == 0), stop=(d == ND - 1))

                def vo_copy():
                    nc.vector.tensor_copy(out=v_sb[:, 4 * w : 4 * w + 4, 0:H],
                                          in_=vo_t[w][:, :, 0:H])

                for d in range(ND):
                    units.append(lambda d=d: qk_d(d))
                units.append(qk_copy)
                for u in range(4):
                    units.append(lambda u=u: vo_u(u))
                units.append(vo_copy)
                return units

            def kv_units(w, prefetch_wave=None):
                """[Wk|Wv] over partner wave w (+ optional next-wave DMA)."""
                osl = slice(2048 + w * 512, 2048 + (w + 1) * 512)
                st = {}
                units = []
                if prefetch_wave is not None:
                    units.append(lambda: emit_wave_dma(prefetch_wave))

                def kv_d(d):
                    if d == 0:
                        st["kv"] = psW.tile([128, 512], f32, tag="wps", name="wps")
                    nc.tensor.matmul(st["kv"], lhsT=wkv_sb[:, d * 128 : (d + 1) * 128],
                                     rhs=xq_t[w][:, d * 512 : (d + 1) * 512],
                                     start=(d == 0), stop=(d == ND - 1))

                def kv_copy():
                    nc.vector.tensor_copy(out=kT[:, osl], in_=st["kv"][0:64, :])
                    st["vt"] = vtt.tile([64, 512], bf16, tag="vt", name="vt")
                    nc.vector.tensor_copy(out=st["vt"], in_=st["kv"][64:128, :])

                def tp_all():
                    # transpose scratch shares the (idle-between-waves) vo
                    # bank via a bf16 view: PSUM accumulation zones allow only
                    # one pending group per 2KB bank, so no dedicated bank
                    tp = vo_t[w][:, :, 0:32].bitcast(bf16)  # [128, 4, 64]
                    for u in range(4):
                        nc.tensor.transpose(tp[:, u, :],
                                            st["vt"][:, u * 128 : (u + 1) * 128],
                                            ident[0:64, 0:64])
                    nc.vector.tensor_copy(
                        out=v_sb[:, NOWN + 4 * w : NOWN + 4 * w + 4, 0:H],
                        in_=tp)

                for d in range(ND):
                    units.append(lambda d=d: kv_d(d))
                units.append(kv_copy)
                units.append(tp_all)
                return units

            def run_span(j, m_own, m_par, drain_to, inject_point):
                """Attention span j (256 queries = own query tiles 2j, 2j+1;
                qT cols [256j, 256j+256)). PSUM accumulation allows one
                pending group per 2KB bank, so the two per-qtile accumulators
                live in two separate banks. Projection-stream units are
                injected between score matmuls; `m_own`/`m_par` are stream
                milestones that must be drained before own/partner chunks."""
                qsl0 = j * 256
                # chunk descriptors: (kT pos, v slot, u0, masked)
                chunks = []
                for c in range(2 * j + 2):          # own
                    u0 = max(0, c - 2 * j)
                    chunks.append((c * 128, c, u0, c >= 2 * j, 0))
                for c in range(2 * j + 2):          # partner
                    u0 = max(0, c - 2 * j)
                    chunks.append((2048 + c * 128, NOWN + c, u0, c >= 2 * j, 1))
                nch = len(chunks)

                # pack chunks into score groups of <=1024 cols (not crossing
                # the own/partner boundary); one exp instruction per group
                groups = []
                cur, wsum = [], 0
                for ci, ch in enumerate(chunks):
                    wdt = 256 - ch[2] * 128
                    if wsum + wdt > 1024 or (ci == nch // 2 and cur):
                        groups.append(cur)
                        cur, wsum = [], 0
                    cur.append((ch, ci, wsum, wdt))
                    wsum += wdt
                groups.append(cur)

                op_ps = [psO0.tile([128, 128], f32, tag="op0", name="op0"),
                         psO1.tile([128, 128], f32, tag="op1", name="op1")]

                def normalize(u):
                    r_sb = rpool.tile([128, 1], f32, tag="r", name="rt")
                    nc.vector.reciprocal(r_sb, op_ps[u][:, H : H + 1])
                    nc.vector.tensor_scalar_mul(
                        out=out_sb[:, 2 * j + u, :],
                        in0=op_ps[u][:, 0:H], scalar1=r_sb)

                def emit_S(grp, inject):
                    """Score matmuls for one group, injecting between chunks."""
                    sc_ps = psS.tile([128, 1024], f32, tag="sc", bufs=2, name="sc")
                    for (pos, slot, u0, msk, mi), ci, off, wdt in grp:
                        nc.tensor.matmul(
                            sc_ps[:, off : off + wdt],
                            lhsT=kT[:, pos : pos + 128],
                            rhs=qT[:, qsl0 + u0 * 128 : qsl0 + 256],
                            start=True, stop=True)
                        inject()
                    return sc_ps

                def emit_exp_mask(sc_ps, grp):
                    p_sb = ppool.tile([128, 1024], bf16, tag="p", name="pt")
                    wtot = grp[-1][2] + grp[-1][3]
                    nc.scalar.activation(out=p_sb[:, 0:wtot], in_=sc_ps[:, 0:wtot],
                                         func=mybir.ActivationFunctionType.Exp,
                                         scale=0.125)
                    for (pos, slot, u0, msk, mi), ci, off, wdt in grp:
                        if msk:
                            nc.vector.tensor_mul(
                                out=p_sb[:, off : off + 128],
                                in0=p_sb[:, off : off + 128],
                                in1=mask_sb[:, mi * 128 : (mi + 1) * 128])
                    return p_sb

                def emit_PV(p_sb, grp):
                    for (pos, slot, u0, msk, mi), ci, off, wdt in grp:
                        c = (slot - NOWN) if mi else slot
                        for u in range(u0, 2):
                            # qtile u complete at partner chunk c == 2j+u
                            is_stop = (mi == 1 and c == 2 * j + u)
                            nc.tensor.matmul(
                                op_ps[u][:, 0 : H + 1],
                                lhsT=p_sb[:, off + (u - u0) * 128 : off + (u - u0 + 1) * 128],
                                rhs=v_sb[:, slot, :],
                                start=(ci == 0), stop=is_stop)
                            if is_stop:
                                normalize(u)

                nown_g = sum(1 for g in groups if g[0][0][4] == 0)
                drain_to(m_own)
                pend = [None]
                for gi, grp in enumerate(groups):
                    if gi == nown_g:
                        drain_to(m_par)
                    sc_ps = emit_S(grp, inject_point)
                    p_sb = emit_exp_mask(sc_ps, grp)
                    if pend[0] is not None:
                        emit_PV(*pend[0])
                    pend[0] = (p_sb, grp)
                emit_PV(*pend[0])

                nc.sync.dma_start(
                    out=out[2 * j : 2 * j + 2].rearrange("c p h -> p c h"),
                    in_=out_sb[:, 2 * j : 2 * j + 2, :])

            # ---- prologue: DMAs in latency-critical order ----
            xo0 = xwp.tile([128, ND * 512], bf16, tag="xw", name="xwt")
            nc.sync.dma_start(out=xo0[:, 0 : 4 * 512], in_=xw[:, 0:1, 0 : 4 * 512])
            nc.sync.dma_start(out=xo0[:, 4 * 512 :], in_=xw[:, 0:1, 4 * 512 :])
            nc.sync.dma_start(out=wqk_sb, in_=wqk.ap())
            nc.sync.dma_start(out=wv_sb, in_=wv.ap())
            xq0 = xwp.tile([128, ND * 512], bf16, tag="xw", name="xwt")
            nc.sync.dma_start(out=xq0, in_=xw[:, 1:2, :])
            nc.sync.dma_start(out=mask_sb, in_=masks.ap())
            nc.sync.dma_start(out=wkv_sb, in_=wkv.ap())
            xo_t[0], xq_t[0] = xo0, xq0
            emit_wave_dma(1)
            # PE p-state warm-up: identity matmuls with no data deps keep the
            # PE continuously busy through its frequency ramp so the first
            # real projections run at full clock.
            scj = None
            for i in range(NJUNK):
                if i % 8 == 0:
                    scj = psS.tile([128, 1024], f32, tag="sc", bufs=2, name="sc")
                nc.tensor.matmul(scj[:, (i % 8) * 128 : (i % 8 + 1) * 128],
                                 lhsT=ident, rhs=ident, start=True, stop=True)
            for unit in qk_units(0):
                unit()
            # ---- projection stream + milestones ----
            # One global ordered queue of projection units, drained uniformly
            # at score-chunk emission points; spans force-drain to their
            # data-dependency milestones.
            stream = []
            stream.extend(kv_units(0))
            m_kv0 = len(stream)
            stream.append(lambda: emit_wave_dma(2))
            stream.extend(qk_units(1))
            m_qk1 = len(stream)
            stream.extend(kv_units(1))
            m_kv1 = len(stream)
            stream.append(lambda: emit_wave_dma(3))
            stream.extend(qk_units(2))
            m_qk2 = len(stream)
            stream.extend(kv_units(2))
            m_kv2 = len(stream)
            stream.extend(qk_units(3))
            m_qk3 = len(stream)
            stream.extend(kv_units(3))
            m_kv3 = len(stream)
            miles_own = {0: 0, 1: 0, 2: m_qk1, 3: m_qk1,
                         4: m_qk2, 5: m_qk2, 6: m_qk3, 7: m_qk3}
            miles_par = {0: m_kv0, 1: m_kv0, 2: m_kv1, 3: m_kv1,
                         4: m_kv2, 5: m_kv2, 6: m_kv3, 7: m_kv3}
            spos = [0]
            gpoints = [0]
            TOTAL_POINTS = sum(2 * (2 * j + 2) for j in range(8))

            def drain_to(idx):
                while spos[0] < idx:
                    stream[spos[0]]()
                    spos[0] += 1

            def inject_point():
                gpoints[0] += 1
                drain_to(min(len(stream),
                             len(stream) * gpoints[0] // TOTAL_POINTS))

            # ---- spans ----
            for j in range(8):
                run_span(j, miles_own[j], miles_par[j],
                         drain_to, inject_point)
            drain_to(len(stream))
    _split_multi_waits(nc)
    return nc


def _host_inputs(x, Wk, Wq, Wv):
    """Build the 8 per-core input maps (bf16)."""
    maps = []
    wqk = np.concatenate([Wq, Wk], axis=1).astype(BF16)          # [1024,128]
    wkv = np.concatenate([Wk, Wv], axis=1).astype(BF16)
    wqk_r = np.ascontiguousarray(
        wqk.reshape(ND, 128, 128).transpose(1, 0, 2).reshape(128, ND * 128))
    wkv_r = np.ascontiguousarray(
        wkv.reshape(ND, 128, 128).transpose(1, 0, 2).reshape(128, ND * 128))
    wv_r = np.ascontiguousarray(
        Wv.astype(BF16).reshape(ND, 128, H).transpose(1, 0, 2).reshape(128, ND * H))
    s = np.arange(128)[:, None]
    t = np.arange(128)[None, :]
    for c in range(8):
        b, p = c // 2, c % 2
        own = [2 * i + p for i in range(NOWN)]
        oth = [2 * i + (1 - p) for i in range(NOWN)]
        own_rows = np.concatenate([np.arange(g * 128, (g + 1) * 128) for g in own])
        oth_rows = np.concatenate([np.arange(g * 128, (g + 1) * 128) for g in oth])
        xb = x[b].astype(BF16)
        xwc = np.zeros((128, 2 * NW, ND * 512), BF16)
        for w in range(NW):
            for half, rows in ((0, own_rows), (1, oth_rows)):
                blk = xb[rows[w * 512 : (w + 1) * 512]]       # [512, 1024]
                xwc[:, 2 * w + half] = (
                    blk.T.reshape(ND, 128, 512).transpose(1, 0, 2)
                    .reshape(128, ND * 512))
        mk = np.zeros((128, 2, 128), np.float32)
        mk[:, 0] = (s <= t)                  # own diagonal: within-tile causal
        mk[:, 1] = float(p)                  # partner diagonal: all-0 / all-1
        maps.append({"xw": np.ascontiguousarray(xwc), "wqk": wqk_r,
                     "wkv": wkv_r, "wv": wv_r,
                     "masks": mk.reshape(128, 256).astype(BF16)})
    return maps


def kernel(x, Wk, Wq, Wv):
    global _PROG, LAST_EXEC_TIME_NS, LAST_RESULTS
    from concourse.bass_utils import run_bass_kernel_spmd

    if _PROG is None:
        _PROG = _build_program()
    in_maps = _host_inputs(np.asarray(x, np.float32), np.asarray(Wk, np.float32),
                           np.asarray(Wq, np.float32), np.asarray(Wv, np.float32))
    res = run_bass_kernel_spmd(_PROG, in_maps, list(range(8)))
    LAST_EXEC_TIME_NS = res.exec_time_ns
    LAST_RESULTS = res
    out = np.zeros((B, T, H), np.float32)
    for c in range(8):
        b, p = c // 2, c % 2
        oc = np.asarray(res.results[c]["out"]).astype(np.float32).reshape(T // 2, H)
        for i in range(NOWN):
            g = 2 * i + p
            out[b, g * 128 : (g + 1) * 128] = oc[i * 128 : (i + 1) * 128]
    return out
